# revision 32
# baseline (speedup 1.0000x reference)
"""ClusterisedMLP (MoE routing) Trainium2 kernel.

Strategy: data-parallel over the N points axis across 8 NeuronCores, with
host-side routing. Points are bucketed by cluster id; each cluster's points
are split evenly over the 8 cores, and each (core, cluster) bucket is padded
to a common capacity CAP (computed from the actual data, so the compiled
program is SPMD-identical across cores). Each core runs its 8 buckets through
the matching expert's 5-layer MLP:

    enc = [sin, cos] positional encoding (36 features)   -- ScalarE Sin LUT
    h   = relu(enc @ W_in^T + b_in)                      -- PE + ACT/DVE
    h   = relu(h @ W_mid^T + b_mid)   x3
    y   = tanh(h @ W_out^T + b_out)

Weight normalization (torch weight_norm) and the angle range reduction for
the Sin LUT (valid only within ~[-pi, pi]) are done on the host; matmuls run
in float32r (tf32-like, ~1e-4 end-to-end error) with fp32 PSUM accumulation.
Activations are written as float32r by ScalarE/VectorE so they feed the next
matmul directly. All Sin ops run before the MLP so the ACT table set is
loaded exactly twice; chunks are processed in two interleaved streams so the
PE never stalls on ReLU latency.
"""
import os
import sys

for _p in ("/opt/trn_rl_repo", "/root/.axon_site/_ro/trn_rl_repo"):
    if _p not in sys.path:
        sys.path.append(_p)

import numpy as np
import ml_dtypes

import concourse.bass as bass
import concourse.tile as tile
from concourse import bacc, mybir
from concourse.bass_utils import run_bass_kernel_spmd

F32 = mybir.dt.float32
F32R = mybir.dt.float32r
BF16 = mybir.dt.bfloat16
AF = mybir.ActivationFunctionType
ALU = mybir.AluOpType

NCORES = 8
C = 8          # clusters / experts
H = 256        # hidden width
DIN = 36       # positional-encoding features
NFREQ = 6
LMID = 3
TILE_N = 512   # points per matmul (one fp32 PSUM bank)

MM_DTYPE = os.environ.get("BASS_MM_DTYPE", "f32r")  # f32r | bf16

def _relu_on_act(bucket, layer, mh):
    """Engine plan: bucket 0 runs all relus on DVE (ACT is busy with the
    pipelined Sin batch); later buckets give ACT the mh=0 half of the mid
    layers (3 ACT / 5 DVE halves per chunk)."""
    return bucket != 0 and layer >= 1 and mh == 0

_prog_cache = {}
_last_in_maps = None


class _Stream:
    """Per-chunk pipeline state."""
    def __init__(self, c, off, n):
        self.c, self.off, self.n = c, off, n


def _bucket_chunks(cap):
    """Split cap into chunks of <=512. A small trailing remainder is merged
    with the last full chunk and split evenly (rounded to 32) so no chunk is
    tiny — tiny chunks make sparse PE phases that trip the HAM clock gate."""
    sizes = []
    left = cap
    while left > 0:
        n = min(TILE_N, left)
        sizes.append(n)
        left -= n
    if len(sizes) >= 2 and sizes[-1] < 256:
        merged = sizes[-2] + sizes[-1]
        a = (merged // 2 + 31) // 32 * 32
        sizes[-2:] = [a, merged - a]
    out = []
    off = 0
    for n in sizes:
        out.append((off, n))
        off += n
    return out


def _build_program(cap):
    """Build + compile the SPMD program for per-bucket capacity `cap`."""
    MMD = F32R if MM_DTYPE == "f32r" else BF16
    nc = bacc.Bacc("TRN2", target_bir_lowering=False, debug=False,
                   num_devices=NCORES)

    xrep_d = nc.dram_tensor("xrep", [DIN, C * cap], F32, kind="ExternalInput").ap()
    winT_d = nc.dram_tensor("winT", [DIN, C, H], MMD, kind="ExternalInput").ap()
    bin_d = nc.dram_tensor("bin", [128, C, 2], F32, kind="ExternalInput").ap()
    wmidT_d = nc.dram_tensor("wmidT", [128, C, LMID, 2, 2, 128], MMD,
                             kind="ExternalInput").ap()
    bmid_d = nc.dram_tensor("bmid", [128, C, LMID, 2], F32, kind="ExternalInput").ap()
    woutT_d = nc.dram_tensor("woutT", [128, C, 2, 3], MMD, kind="ExternalInput").ap()
    bout_d = nc.dram_tensor("bout", [128, C], F32, kind="ExternalInput").ap()
    yT_d = nc.dram_tensor("yT", [3, C * cap], F32, kind="ExternalOutput").ap()

    with tile.TileContext(nc) as tc:
        with tc.tile_pool(name="wp", bufs=1) as wp, \
             tc.tile_pool(name="xp", bufs=2) as xp, \
             tc.tile_pool(name="hp", bufs=9) as hp, \
             tc.tile_pool(name="yp", bufs=2) as yp, \
             tc.tile_pool(name="psh", bufs=3, space="PSUM") as psh, \
             tc.tile_pool(name="pso", bufs=2, space="PSUM") as pso:

            # One HWDGE ring, DMAs ordered by first-use time in the pipeline.
            xc_tiles = [xp.tile([DIN, cap], F32, tag="xc", name=f"xc{c}",
                                bufs=8) for c in range(C)]
            winT_sb = wp.tile([DIN, C, H], MMD)
            bin_sb = wp.tile([128, C, 2], F32)
            bmid_sb = wp.tile([128, C, LMID, 2], F32)
            woutT_sb = wp.tile([128, C, 2, 3], MMD)
            bout_sb = wp.tile([128, C], F32)
            wmidT_sb = wp.tile([128, C, LMID, 2, 2, 128], MMD)

            # x data for buckets 1..7 on the ACT HWDGE ring (small transfers,
            # land early -> Sin ops stay ready and keep their early schedule
            # slot); weights stream on the SP ring.
            for c in range(1, C):
                nc.scalar.dma_start(xc_tiles[c][:],
                                    xrep_d[:, c * cap:(c + 1) * cap])
            nc.sync.dma_start(winT_sb[:], winT_d[:])
            for off, n in _bucket_chunks(cap):  # bucket 0 x data in pieces
                nc.sync.dma_start(xc_tiles[0][:, off:off + n],
                                  xrep_d[:, off:off + n])
            nc.sync.dma_start(bin_sb[:], bin_d[:])
            nc.sync.dma_start(bmid_sb[:], bmid_d[:])
            nc.sync.dma_start(wmidT_sb[:, 0], wmidT_d[:, 0])
            nc.sync.dma_start(woutT_sb[:], woutT_d[:])
            nc.sync.dma_start(bout_sb[:], bout_d[:])
            for c in range(1, C):
                nc.sync.dma_start(wmidT_sb[:, c], wmidT_d[:, c])

            enc_all = wp.tile([DIN, C * cap], MMD)

            # dummy Sin with no DMA dependency: pulls the ACT table load
            # forward so the first real Sin isn't gated on it
            dummy = wp.tile([1, 8], F32)
            nc.vector.memset(dummy[:], 0.0)
            nc.scalar.activation(dummy[:], dummy[:], AF.Sin)

            def sin_bucket(c):
                nc.scalar.activation(enc_all[:, c * cap:(c + 1) * cap],
                                     xc_tiles[c][:], AF.Sin)

            # bucket 0's encodings chunk-by-chunk so its first matmul starts
            # as early as possible; later buckets' Sins are injected into
            # bucket 0's stage stream below (relu shares Sin's table set).
            for off, n in _bucket_chunks(cap):
                nc.scalar.activation(enc_all[:, off:off + n],
                                     xc_tiles[0][:, off:off + n], AF.Sin)

            # ---- per bucket: chunks as concurrently-pipelined streams ----
            # The output layer of bucket c is deferred into bucket c+1's
            # dense Lin/mid phases so PE activity never dips at bucket
            # boundaries (a dip re-throttles the PE clock via HAM).
            deferred = []   # list of (c, stream) awaiting out-stage emission

            def emit_out(count):
                for _ in range(min(count, len(deferred))):
                    cc, s, yc_c = deferred.pop(0)
                    po = pso.tile([128, TILE_N], F32, tag="po", name="po")
                    for kh in range(2):
                        nc.tensor.matmul(po[0:3, :s.n], woutT_sb[:, cc, kh],
                                         s.h[:, kh, :s.n],
                                         start=(kh == 0), stop=(kh == 1))
                    nc.scalar.activation(yc_c[0:3, s.off:s.off + s.n],
                                         po[0:3, :s.n], AF.Tanh,
                                         bias=bout_sb[0:3, cc:cc + 1])
                    nc.sync.dma_start(
                        yT_d[:, cc * cap + s.off:cc * cap + s.off + s.n],
                        yc_c[0:3, s.off:s.off + s.n])

            for c in range(C):
                streams = [_Stream(c, off, n) for off, n in _bucket_chunks(cap)]
                yc = yp.tile([128, cap], F32, tag="yc", name=f"yc{c}")
                for s in streams:
                    s.enc = enc_all[:, c * cap + s.off:c * cap + s.off + s.n]

                def inject_sin(stage):
                    # pipeline later buckets' Sin ops through bucket 0
                    if c == 0:
                        for cc in range(1 + 2 * stage, 1 + 2 * (stage + 1)):
                            if cc < C:
                                sin_bucket(cc)

                # input layer: K=36 -> [256, n]
                for s in streams:
                    s.ps = psh.tile([128, 2, TILE_N], F32, tag="ps", name="psA")
                for s in streams:
                    for mh in range(2):
                        nc.tensor.matmul(
                            s.ps[:, mh, :s.n],
                            winT_sb[:, c, mh * 128:(mh + 1) * 128],
                            s.enc, start=True, stop=True)
                inject_sin(0)
                emit_out(2)
                for s in streams:
                    s.h = hp.tile([128, 2, TILE_N], MMD, tag="h", name="hA")
                    _relu(nc, c, 0, s.h, s.ps, bin_sb[:, c], s.n)

                # mid layers: K=256 -> [256, n]
                for l in range(LMID):
                    for s in streams:
                        s.ps = psh.tile([128, 2, TILE_N], F32, tag="ps",
                                        name="psB")
                    for s in streams:
                        for mh in range(2):
                            for kh in range(2):
                                nc.tensor.matmul(
                                    s.ps[:, mh, :s.n],
                                    wmidT_sb[:, c, l, kh, mh],
                                    s.h[:, kh, :s.n],
                                    start=(kh == 0), stop=(kh == 1))
                    inject_sin(1 + l)
                    if l == 0:
                        emit_out(1)
                    for s in streams:
                        s.h = hp.tile([128, 2, TILE_N], MMD, tag="h", name="hB")
                        _relu(nc, c, 1 + l, s.h, s.ps, bmid_sb[:, c, l], s.n)

                for s in streams:
                    deferred.append((c, s, yc))
            emit_out(len(deferred))

    nc.compile()
    return nc


def _relu(nc, bucket, layer, h, ps, b2, n):
    """h[:, mh, :n] = relu(ps[:, mh, :n] + b2[:, mh]) per the engine plan."""
    for mh in range(2):
        dst, src = h[:, mh, :n], ps[:, mh, :n]
        bias = b2[:, mh:mh + 1]
        if _relu_on_act(bucket, layer, mh):
            nc.scalar.activation(dst, src, AF.Relu, bias=bias)
        else:
            nc.vector.tensor_scalar(dst, src, bias, 0.0, ALU.add, ALU.max)


def _wn(V, g):
    return g[:, None] * V / np.linalg.norm(V, axis=1, keepdims=True)


def kernel(X, cluster_ids, V_in, g_in, b_in, V_mid, g_mid, b_mid,
           V_out, g_out, b_out):
    X = np.ascontiguousarray(np.asarray(X, dtype=np.float32))
    cid = np.asarray(cluster_ids).astype(np.int64)
    N = X.shape[0]

    # ---- host: weight norm (fp32, matches reference) ----
    V_in = np.asarray(V_in, np.float32)
    g_in = np.asarray(g_in, np.float32)
    b_in = np.asarray(b_in, np.float32)
    V_mid = np.asarray(V_mid, np.float32)
    g_mid = np.asarray(g_mid, np.float32)
    b_mid = np.asarray(b_mid, np.float32)
    V_out = np.asarray(V_out, np.float32)
    g_out = np.asarray(g_out, np.float32)
    b_out = np.asarray(b_out, np.float32)

    W_in = np.stack([_wn(V_in[c], g_in[c]) for c in range(C)])          # [C,H,DIN]
    W_mid = np.stack([[_wn(V_mid[c, l], g_mid[c, l]) for l in range(LMID)]
                      for c in range(C)])                               # [C,L,H,H]
    W_out = np.stack([_wn(V_out[c], g_out[c]) for c in range(C)])       # [C,3,H]

    # ---- host: routing ----
    sel = []
    counts = np.zeros((NCORES, C), np.int64)
    for c in range(C):
        ii = np.where(cid == c)[0]
        sel.append([ii[j::NCORES] for j in range(NCORES)])
        for j in range(NCORES):
            counts[j, c] = len(sel[c][j])
    cap = int(counts.max())
    cap = max(64, -(-cap // 64) * 64)  # round up to multiple of 64

    wdt = np.float32 if MM_DTYPE == "f32r" else ml_dtypes.bfloat16
    winT = np.ascontiguousarray(W_in.transpose(2, 0, 1)).astype(wdt)    # [36,C,H]
    binh = np.ascontiguousarray(
        b_in.reshape(C, 2, 128).transpose(2, 0, 1))                     # [128,C,2]
    wmidT = np.ascontiguousarray(
        W_mid.reshape(C, LMID, 2, 128, 2, 128)                          # c,l,mh,mp,kh,kp
        .transpose(5, 0, 1, 4, 2, 3)).astype(wdt)                       # [128,C,L,2,2,128]
    bmidh = np.ascontiguousarray(
        b_mid.reshape(C, LMID, 2, 128).transpose(3, 0, 1, 2))           # [128,C,L,2]
    woutT = np.ascontiguousarray(
        W_out.reshape(C, 3, 2, 128).transpose(3, 0, 2, 1)).astype(wdt)  # [128,C,2,3]
    bouth = np.zeros((128, C), np.float32)  # b_out replicated per col strip
    for base in (0, 32, 64):
        bouth[base:base + 3] = b_out.T

    # ---- host: per-core gathered, range-reduced angles [36, C*cap] ----
    freqs = (2.0 ** np.arange(NFREQ)).astype(np.float64)                # [6]
    in_maps = []
    for j in range(NCORES):
        xrep = np.zeros((DIN, C * cap), np.float32)
        for c in range(C):
            ii = sel[c][j]
            if len(ii) == 0:
                continue
            xg = X[ii].astype(np.float64)                               # [m,3]
            ang = xg[:, None, :] * freqs[None, :, None]                 # [m,6,3]
            ang = np.concatenate([ang, ang + np.pi / 2], axis=2)        # [m,6,6]
            ang = np.mod(ang + np.pi, 2 * np.pi) - np.pi                # [-pi, pi)
            xrep[:, c * cap:c * cap + len(ii)] = \
                ang.reshape(len(ii), DIN).T.astype(np.float32)
        in_maps.append(dict(xrep=xrep, winT=winT, bin=binh, wmidT=wmidT,
                            bmid=bmidh, woutT=woutT, bout=bouth))

    # ---- device ----
    global _last_in_maps
    _last_in_maps = in_maps
    key = (cap, MM_DTYPE)
    if key not in _prog_cache:
        _prog_cache[key] = _build_program(cap)
    nc = _prog_cache[key]
    res = run_bass_kernel_spmd(nc, in_maps, core_ids=list(range(NCORES)))

    # ---- host: scatter back ----
    out = np.zeros((N, 3), np.float32)
    for j in range(NCORES):
        yT = res.results[j]["yT"]                                       # [3, C*cap]
        for c in range(C):
            ii = sel[c][j]
            if len(ii):
                out[ii] = yT[:, c * cap:c * cap + len(ii)].T
    return out


# revision 33
# speedup vs baseline: 1.0626x; 1.0626x over previous
"""ClusterisedMLP (MoE routing) Trainium2 kernel.

Strategy: data-parallel over the N points axis across 8 NeuronCores, with
host-side routing. Points are bucketed by cluster id; each cluster's points
are split evenly over the 8 cores, and each (core, cluster) bucket is padded
to a common capacity CAP (computed from the actual data, so the compiled
program is SPMD-identical across cores). Each core runs its 8 buckets through
the matching expert's 5-layer MLP:

    enc = [sin, cos] positional encoding (36 features)   -- ScalarE Sin LUT
    h   = relu(enc @ W_in^T + b_in)                      -- PE + ACT/DVE
    h   = relu(h @ W_mid^T + b_mid)   x3
    y   = tanh(h @ W_out^T + b_out)

Weight normalization (torch weight_norm) and the angle range reduction for
the Sin LUT (valid only within ~[-pi, pi]) are done on the host; matmuls run
in float32r (tf32-like, ~1e-4 end-to-end error) with fp32 PSUM accumulation.
Activations are written as float32r by ScalarE/VectorE so they feed the next
matmul directly. All Sin ops run before the MLP so the ACT table set is
loaded exactly twice; chunks are processed in two interleaved streams so the
PE never stalls on ReLU latency.
"""
import os
import sys

for _p in ("/opt/trn_rl_repo", "/root/.axon_site/_ro/trn_rl_repo"):
    if _p not in sys.path:
        sys.path.append(_p)

import numpy as np
import ml_dtypes

import concourse.bass as bass
import concourse.tile as tile
from concourse import bacc, mybir
from concourse.bass_utils import run_bass_kernel_spmd

F32 = mybir.dt.float32
F32R = mybir.dt.float32r
BF16 = mybir.dt.bfloat16
AF = mybir.ActivationFunctionType
ALU = mybir.AluOpType

NCORES = 8
C = 8          # clusters / experts
H = 256        # hidden width
DIN = 36       # positional-encoding features
NFREQ = 6
LMID = 3
TILE_N = 512   # points per matmul (one fp32 PSUM bank)

MM_DTYPE = os.environ.get("BASS_MM_DTYPE", "f32r")  # f32r | bf16

def _relu_on_act(bucket, layer, mh):
    """Engine plan: bucket 0 runs all relus on DVE (ACT is busy with the
    pipelined Sin batch); later buckets give ACT the mh=0 half of the mid
    layers (3 ACT / 5 DVE halves per chunk)."""
    return bucket != 0 and layer >= 1 and mh == 0

_prog_cache = {}
_last_in_maps = None


class _Stream:
    """Per-chunk pipeline state."""
    def __init__(self, c, off, n):
        self.c, self.off, self.n = c, off, n


def _bucket_chunks(cap):
    """Split cap into chunks of <=512. A small trailing remainder is merged
    with the last full chunk and split evenly (rounded to 32) so no chunk is
    tiny — tiny chunks make sparse PE phases that trip the HAM clock gate."""
    sizes = []
    left = cap
    while left > 0:
        n = min(TILE_N, left)
        sizes.append(n)
        left -= n
    if len(sizes) >= 2 and sizes[-1] < 256:
        merged = sizes[-2] + sizes[-1]
        a = (merged // 2 + 31) // 32 * 32
        sizes[-2:] = [a, merged - a]
    out = []
    off = 0
    for n in sizes:
        out.append((off, n))
        off += n
    return out


def _build_program(cap):
    """Build + compile the SPMD program for per-bucket capacity `cap`."""
    MMD = F32R if MM_DTYPE == "f32r" else BF16
    nc = bacc.Bacc("TRN2", target_bir_lowering=False, debug=False,
                   num_devices=NCORES)

    xrep_d = nc.dram_tensor("xrep", [DIN, C * cap], F32, kind="ExternalInput").ap()
    winT_d = nc.dram_tensor("winT", [DIN, C, H], MMD, kind="ExternalInput").ap()
    bin_d = nc.dram_tensor("bin", [128, C, 2], F32, kind="ExternalInput").ap()
    wmidT_d = nc.dram_tensor("wmidT", [128, C, LMID, 2, 2, 128], MMD,
                             kind="ExternalInput").ap()
    bmid_d = nc.dram_tensor("bmid", [128, C, LMID, 2], F32, kind="ExternalInput").ap()
    woutT_d = nc.dram_tensor("woutT", [128, C, 2, 3], MMD, kind="ExternalInput").ap()
    bout_d = nc.dram_tensor("bout", [128, C], F32, kind="ExternalInput").ap()
    yT_d = nc.dram_tensor("yT", [3, C * cap], F32, kind="ExternalOutput").ap()

    with tile.TileContext(nc) as tc:
        with tc.tile_pool(name="wp", bufs=1) as wp, \
             tc.tile_pool(name="xp", bufs=2) as xp, \
             tc.tile_pool(name="hp", bufs=9) as hp, \
             tc.tile_pool(name="yp", bufs=2) as yp, \
             tc.tile_pool(name="psh", bufs=3, space="PSUM") as psh, \
             tc.tile_pool(name="pso", bufs=2, space="PSUM") as pso:

            # One HWDGE ring, DMAs ordered by first-use time in the pipeline.
            xc_tiles = [xp.tile([DIN, cap], F32, tag="xc", name=f"xc{c}",
                                bufs=8) for c in range(C)]
            winT_sb = wp.tile([DIN, C, H], MMD)
            bin_sb = wp.tile([128, C, 2], F32)
            bmid_sb = wp.tile([128, C, LMID, 2], F32)
            woutT_sb = wp.tile([128, C, 2, 3], MMD)
            bout_sb = wp.tile([128, C], F32)
            wmidT_sb = wp.tile([128, C, LMID, 2, 2, 128], MMD)

            # All DMAs on the SP ring, ordered by first use: x data early so
            # every Sin is ready long before the scheduler places it (late
            # Sins get reordered behind Tanh/Relu -> ACT table thrashing).
            nc.sync.dma_start(winT_sb[:], winT_d[:])
            for off, n in _bucket_chunks(cap):  # bucket 0 x data in pieces
                nc.sync.dma_start(xc_tiles[0][:, off:off + n],
                                  xrep_d[:, off:off + n])
            nc.sync.dma_start(bin_sb[:], bin_d[:])
            nc.sync.dma_start(bmid_sb[:], bmid_d[:])
            for c in range(1, C):
                nc.sync.dma_start(xc_tiles[c][:],
                                  xrep_d[:, c * cap:(c + 1) * cap])
            nc.sync.dma_start(wmidT_sb[:, 0], wmidT_d[:, 0])
            nc.sync.dma_start(woutT_sb[:], woutT_d[:])
            nc.sync.dma_start(bout_sb[:], bout_d[:])
            for c in range(1, C):
                nc.sync.dma_start(wmidT_sb[:, c], wmidT_d[:, c])

            enc_all = wp.tile([DIN, C * cap], MMD)

            # dummy Sin with no DMA dependency: pulls the ACT table load
            # forward so the first real Sin isn't gated on it
            dummy = wp.tile([1, 8], F32)
            nc.vector.memset(dummy[:], 0.0)
            nc.scalar.activation(dummy[:], dummy[:], AF.Sin)

            def sin_bucket(c):
                nc.scalar.activation(enc_all[:, c * cap:(c + 1) * cap],
                                     xc_tiles[c][:], AF.Sin)

            # bucket 0's encodings chunk-by-chunk so its first matmul starts
            # as early as possible; later buckets' Sins are injected into
            # bucket 0's stage stream below (relu shares Sin's table set).
            for off, n in _bucket_chunks(cap):
                nc.scalar.activation(enc_all[:, off:off + n],
                                     xc_tiles[0][:, off:off + n], AF.Sin)

            # ---- per bucket: chunks as concurrently-pipelined streams ----
            # The output layer of bucket c is deferred into bucket c+1's
            # dense Lin/mid phases so PE activity never dips at bucket
            # boundaries (a dip re-throttles the PE clock via HAM).
            deferred = []   # list of (c, stream) awaiting out-stage emission

            def emit_out(count):
                for _ in range(min(count, len(deferred))):
                    cc, s, yc_c = deferred.pop(0)
                    po = pso.tile([128, TILE_N], F32, tag="po", name="po")
                    for kh in range(2):
                        nc.tensor.matmul(po[0:3, :s.n], woutT_sb[:, cc, kh],
                                         s.h[:, kh, :s.n],
                                         start=(kh == 0), stop=(kh == 1))
                    nc.scalar.activation(yc_c[0:3, s.off:s.off + s.n],
                                         po[0:3, :s.n], AF.Tanh,
                                         bias=bout_sb[0:3, cc:cc + 1])
                    nc.sync.dma_start(
                        yT_d[:, cc * cap + s.off:cc * cap + s.off + s.n],
                        yc_c[0:3, s.off:s.off + s.n])

            for c in range(C):
                streams = [_Stream(c, off, n) for off, n in _bucket_chunks(cap)]
                yc = yp.tile([128, cap], F32, tag="yc", name=f"yc{c}")
                for s in streams:
                    s.enc = enc_all[:, c * cap + s.off:c * cap + s.off + s.n]

                def inject_sin(stage):
                    # pipeline later buckets' Sin ops through bucket 0
                    if c == 0:
                        for cc in range(1 + 2 * stage, 1 + 2 * (stage + 1)):
                            if cc < C:
                                sin_bucket(cc)

                # input layer: K=36 -> [256, n]
                for s in streams:
                    s.ps = psh.tile([128, 2, TILE_N], F32, tag="ps", name="psA")
                for s in streams:
                    for mh in range(2):
                        nc.tensor.matmul(
                            s.ps[:, mh, :s.n],
                            winT_sb[:, c, mh * 128:(mh + 1) * 128],
                            s.enc, start=True, stop=True)
                inject_sin(0)
                emit_out(2)
                for s in streams:
                    s.h = hp.tile([128, 2, TILE_N], MMD, tag="h", name="hA")
                    _relu(nc, c, 0, s.h, s.ps, bin_sb[:, c], s.n)

                # mid layers: K=256 -> [256, n]
                for l in range(LMID):
                    for s in streams:
                        s.ps = psh.tile([128, 2, TILE_N], F32, tag="ps",
                                        name="psB")
                    for s in streams:
                        for mh in range(2):
                            for kh in range(2):
                                nc.tensor.matmul(
                                    s.ps[:, mh, :s.n],
                                    wmidT_sb[:, c, l, kh, mh],
                                    s.h[:, kh, :s.n],
                                    start=(kh == 0), stop=(kh == 1))
                    inject_sin(1 + l)
                    if l == 0:
                        emit_out(1)
                    for s in streams:
                        s.h = hp.tile([128, 2, TILE_N], MMD, tag="h", name="hB")
                        _relu(nc, c, 1 + l, s.h, s.ps, bmid_sb[:, c, l], s.n)

                for s in streams:
                    deferred.append((c, s, yc))
            emit_out(len(deferred))

    nc.compile()
    return nc


def _relu(nc, bucket, layer, h, ps, b2, n):
    """h[:, mh, :n] = relu(ps[:, mh, :n] + b2[:, mh]) per the engine plan."""
    for mh in range(2):
        dst, src = h[:, mh, :n], ps[:, mh, :n]
        bias = b2[:, mh:mh + 1]
        if _relu_on_act(bucket, layer, mh):
            nc.scalar.activation(dst, src, AF.Relu, bias=bias)
        else:
            nc.vector.tensor_scalar(dst, src, bias, 0.0, ALU.add, ALU.max)


def _wn(V, g):
    return g[:, None] * V / np.linalg.norm(V, axis=1, keepdims=True)


def kernel(X, cluster_ids, V_in, g_in, b_in, V_mid, g_mid, b_mid,
           V_out, g_out, b_out):
    X = np.ascontiguousarray(np.asarray(X, dtype=np.float32))
    cid = np.asarray(cluster_ids).astype(np.int64)
    N = X.shape[0]

    # ---- host: weight norm (fp32, matches reference) ----
    V_in = np.asarray(V_in, np.float32)
    g_in = np.asarray(g_in, np.float32)
    b_in = np.asarray(b_in, np.float32)
    V_mid = np.asarray(V_mid, np.float32)
    g_mid = np.asarray(g_mid, np.float32)
    b_mid = np.asarray(b_mid, np.float32)
    V_out = np.asarray(V_out, np.float32)
    g_out = np.asarray(g_out, np.float32)
    b_out = np.asarray(b_out, np.float32)

    W_in = np.stack([_wn(V_in[c], g_in[c]) for c in range(C)])          # [C,H,DIN]
    W_mid = np.stack([[_wn(V_mid[c, l], g_mid[c, l]) for l in range(LMID)]
                      for c in range(C)])                               # [C,L,H,H]
    W_out = np.stack([_wn(V_out[c], g_out[c]) for c in range(C)])       # [C,3,H]

    # ---- host: routing ----
    sel = []
    counts = np.zeros((NCORES, C), np.int64)
    for c in range(C):
        ii = np.where(cid == c)[0]
        sel.append([ii[j::NCORES] for j in range(NCORES)])
        for j in range(NCORES):
            counts[j, c] = len(sel[c][j])
    cap = int(counts.max())
    cap = max(64, -(-cap // 64) * 64)  # round up to multiple of 64

    wdt = np.float32 if MM_DTYPE == "f32r" else ml_dtypes.bfloat16
    winT = np.ascontiguousarray(W_in.transpose(2, 0, 1)).astype(wdt)    # [36,C,H]
    binh = np.ascontiguousarray(
        b_in.reshape(C, 2, 128).transpose(2, 0, 1))                     # [128,C,2]
    wmidT = np.ascontiguousarray(
        W_mid.reshape(C, LMID, 2, 128, 2, 128)                          # c,l,mh,mp,kh,kp
        .transpose(5, 0, 1, 4, 2, 3)).astype(wdt)                       # [128,C,L,2,2,128]
    bmidh = np.ascontiguousarray(
        b_mid.reshape(C, LMID, 2, 128).transpose(3, 0, 1, 2))           # [128,C,L,2]
    woutT = np.ascontiguousarray(
        W_out.reshape(C, 3, 2, 128).transpose(3, 0, 2, 1)).astype(wdt)  # [128,C,2,3]
    bouth = np.zeros((128, C), np.float32)  # b_out replicated per col strip
    for base in (0, 32, 64):
        bouth[base:base + 3] = b_out.T

    # ---- host: per-core gathered, range-reduced angles [36, C*cap] ----
    freqs = (2.0 ** np.arange(NFREQ)).astype(np.float64)                # [6]
    in_maps = []
    for j in range(NCORES):
        xrep = np.zeros((DIN, C * cap), np.float32)
        for c in range(C):
            ii = sel[c][j]
            if len(ii) == 0:
                continue
            xg = X[ii].astype(np.float64)                               # [m,3]
            ang = xg[:, None, :] * freqs[None, :, None]                 # [m,6,3]
            ang = np.concatenate([ang, ang + np.pi / 2], axis=2)        # [m,6,6]
            ang = np.mod(ang + np.pi, 2 * np.pi) - np.pi                # [-pi, pi)
            xrep[:, c * cap:c * cap + len(ii)] = \
                ang.reshape(len(ii), DIN).T.astype(np.float32)
        in_maps.append(dict(xrep=xrep, winT=winT, bin=binh, wmidT=wmidT,
                            bmid=bmidh, woutT=woutT, bout=bouth))

    # ---- device ----
    global _last_in_maps
    _last_in_maps = in_maps
    key = (cap, MM_DTYPE)
    if key not in _prog_cache:
        _prog_cache[key] = _build_program(cap)
    nc = _prog_cache[key]
    res = run_bass_kernel_spmd(nc, in_maps, core_ids=list(range(NCORES)))

    # ---- host: scatter back ----
    out = np.zeros((N, 3), np.float32)
    for j in range(NCORES):
        yT = res.results[j]["yT"]                                       # [3, C*cap]
        for c in range(C):
            ii = sel[c][j]
            if len(ii):
                out[ii] = yT[:, c * cap:c * cap + len(ii)].T
    return out


# revision 34
# speedup vs baseline: 1.1978x; 1.1272x over previous
"""ClusterisedMLP (MoE routing) Trainium2 kernel.

Strategy: data-parallel over the N points axis across 8 NeuronCores, with
host-side routing. Points are bucketed by cluster id; each cluster's points
are split evenly over the 8 cores, and each (core, cluster) bucket is padded
to a common capacity CAP (computed from the actual data, so the compiled
program is SPMD-identical across cores). Each core runs its 8 buckets through
the matching expert's 5-layer MLP:

    enc = [sin, cos] positional encoding (36 features)   -- ScalarE Sin LUT
    h   = relu(enc @ W_in^T + b_in)                      -- PE + ACT/DVE
    h   = relu(h @ W_mid^T + b_mid)   x3
    y   = tanh(h @ W_out^T + b_out)

Weight normalization (torch weight_norm) and the angle range reduction for
the Sin LUT (valid only within ~[-pi, pi]) are done on the host; matmuls run
in float32r (tf32-like, ~1e-4 end-to-end error) with fp32 PSUM accumulation.
Activations are written as float32r by ScalarE/VectorE so they feed the next
matmul directly. All Sin ops run before the MLP so the ACT table set is
loaded exactly twice; chunks are processed in two interleaved streams so the
PE never stalls on ReLU latency.
"""
import os
import sys

for _p in ("/opt/trn_rl_repo", "/root/.axon_site/_ro/trn_rl_repo"):
    if _p not in sys.path:
        sys.path.append(_p)

import numpy as np
import ml_dtypes

import concourse.bass as bass
import concourse.tile as tile
from concourse import bacc, mybir
from concourse.bass_utils import run_bass_kernel_spmd

F32 = mybir.dt.float32
F32R = mybir.dt.float32r
BF16 = mybir.dt.bfloat16
AF = mybir.ActivationFunctionType
ALU = mybir.AluOpType

NCORES = 8
C = 8          # clusters / experts
H = 256        # hidden width
DIN = 36       # positional-encoding features
NFREQ = 6
LMID = 3
TILE_N = 512   # points per matmul (one fp32 PSUM bank)

MM_DTYPE = os.environ.get("BASS_MM_DTYPE", "f32r")  # f32r | bf16

def _relu_on_act(bucket, layer, mh):
    """Engine plan: bucket 0 runs all relus on DVE (ACT is busy with the
    pipelined Sin batch); later buckets give ACT the mh=0 half of the mid
    layers (3 ACT / 5 DVE halves per chunk)."""
    return bucket != 0 and layer >= 1 and mh == 0

_prog_cache = {}
_last_in_maps = None


class _Stream:
    """Per-chunk pipeline state."""
    def __init__(self, c, off, n):
        self.c, self.off, self.n = c, off, n


def _bucket_chunks(cap):
    """Split cap into chunks of <=512. A small trailing remainder is merged
    with the last full chunk and split evenly (rounded to 32) so no chunk is
    tiny — tiny chunks make sparse PE phases that trip the HAM clock gate."""
    sizes = []
    left = cap
    while left > 0:
        n = min(TILE_N, left)
        sizes.append(n)
        left -= n
    if len(sizes) >= 2 and sizes[-1] < 256:
        merged = sizes[-2] + sizes[-1]
        a = (merged // 2 + 31) // 32 * 32
        sizes[-2:] = [a, merged - a]
    out = []
    off = 0
    for n in sizes:
        out.append((off, n))
        off += n
    return out


def _build_program(cap):
    """Build + compile the SPMD program for per-bucket capacity `cap`."""
    MMD = F32R if MM_DTYPE == "f32r" else BF16
    nc = bacc.Bacc("TRN2", target_bir_lowering=False, debug=False,
                   num_devices=NCORES)

    xrep_d = nc.dram_tensor("xrep", [DIN, C * cap], F32, kind="ExternalInput").ap()
    winT_d = nc.dram_tensor("winT", [DIN, C, H], MMD, kind="ExternalInput").ap()
    bin_d = nc.dram_tensor("bin", [128, C, 2], F32, kind="ExternalInput").ap()
    wmidT_d = nc.dram_tensor("wmidT", [128, C, LMID, 2, 2, 128], MMD,
                             kind="ExternalInput").ap()
    bmid_d = nc.dram_tensor("bmid", [128, C, LMID, 2], F32, kind="ExternalInput").ap()
    woutT_d = nc.dram_tensor("woutT", [128, C, 2, 3], MMD, kind="ExternalInput").ap()
    bout_d = nc.dram_tensor("bout", [128, C], F32, kind="ExternalInput").ap()
    yT_d = nc.dram_tensor("yT", [3, C * cap], F32, kind="ExternalOutput").ap()

    with tile.TileContext(nc) as tc:
        with tc.tile_pool(name="wp", bufs=1) as wp, \
             tc.tile_pool(name="xp", bufs=2) as xp, \
             tc.tile_pool(name="hp", bufs=9) as hp, \
             tc.tile_pool(name="yp", bufs=2) as yp, \
             tc.tile_pool(name="psh", bufs=3, space="PSUM") as psh, \
             tc.tile_pool(name="pso", bufs=2, space="PSUM") as pso:

            # One HWDGE ring, DMAs ordered by first-use time in the pipeline.
            xc_tiles = [xp.tile([DIN, cap], F32, tag="xc", name=f"xc{c}",
                                bufs=8) for c in range(C)]
            winT_sb = wp.tile([DIN, C, H], MMD)
            bin_sb = wp.tile([128, C, 2], F32)
            bmid_sb = wp.tile([128, C, LMID, 2], F32)
            woutT_sb = wp.tile([128, C, 2, 3], MMD)
            bout_sb = wp.tile([128, C], F32)
            wmidT_sb = wp.tile([128, C, LMID, 2, 2, 128], MMD)

            # All DMAs on the SP ring, ordered by first use: x data early so
            # every Sin is ready long before the scheduler places it (late
            # Sins get reordered behind Tanh/Relu -> ACT table thrashing).
            nc.sync.dma_start(winT_sb[:], winT_d[:])
            for off, n in _bucket_chunks(cap):  # bucket 0 x data in pieces
                nc.sync.dma_start(xc_tiles[0][:, off:off + n],
                                  xrep_d[:, off:off + n])
            nc.sync.dma_start(bin_sb[:], bin_d[:])
            nc.sync.dma_start(bmid_sb[:], bmid_d[:])
            nc.sync.dma_start(wmidT_sb[:, 0], wmidT_d[:, 0])
            for c in range(1, C):
                nc.sync.dma_start(xc_tiles[c][:],
                                  xrep_d[:, c * cap:(c + 1) * cap])
            nc.sync.dma_start(woutT_sb[:], woutT_d[:])
            nc.sync.dma_start(bout_sb[:], bout_d[:])
            for c in range(1, C):
                nc.sync.dma_start(wmidT_sb[:, c], wmidT_d[:, c])

            enc_all = wp.tile([DIN, C * cap], MMD)

            # dummy Sin with no DMA dependency: pulls the ACT table load
            # forward so the first real Sin isn't gated on it
            dummy = wp.tile([1, 8], F32)
            nc.vector.memset(dummy[:], 0.0)
            nc.scalar.activation(dummy[:], dummy[:], AF.Sin)

            def sin_bucket(c):
                nc.scalar.activation(enc_all[:, c * cap:(c + 1) * cap],
                                     xc_tiles[c][:], AF.Sin)

            # bucket 0's encodings chunk-by-chunk so its first matmul starts
            # as early as possible; later buckets' Sins are injected into
            # bucket 0's stage stream below (relu shares Sin's table set).
            for off, n in _bucket_chunks(cap):
                nc.scalar.activation(enc_all[:, off:off + n],
                                     xc_tiles[0][:, off:off + n], AF.Sin)

            # ---- per bucket: chunks as concurrently-pipelined streams ----
            # The output layer of bucket c is deferred into bucket c+1's
            # dense Lin/mid phases so PE activity never dips at bucket
            # boundaries (a dip re-throttles the PE clock via HAM).
            deferred = []   # list of (c, stream) awaiting out-stage emission

            def emit_out(count):
                for _ in range(min(count, len(deferred))):
                    cc, s, yc_c = deferred.pop(0)
                    po = pso.tile([128, TILE_N], F32, tag="po", name="po")
                    for kh in range(2):
                        nc.tensor.matmul(po[0:3, :s.n], woutT_sb[:, cc, kh],
                                         s.h[:, kh, :s.n],
                                         start=(kh == 0), stop=(kh == 1))
                    nc.scalar.activation(yc_c[0:3, s.off:s.off + s.n],
                                         po[0:3, :s.n], AF.Tanh,
                                         bias=bout_sb[0:3, cc:cc + 1])
                    nc.sync.dma_start(
                        yT_d[:, cc * cap + s.off:cc * cap + s.off + s.n],
                        yc_c[0:3, s.off:s.off + s.n])

            for c in range(C):
                streams = [_Stream(c, off, n) for off, n in _bucket_chunks(cap)]
                yc = yp.tile([128, cap], F32, tag="yc", name=f"yc{c}")
                for s in streams:
                    s.enc = enc_all[:, c * cap + s.off:c * cap + s.off + s.n]

                def inject_sin(stage):
                    # pipeline later buckets' Sin ops through bucket 0
                    if c == 0:
                        for cc in range(1 + 2 * stage, 1 + 2 * (stage + 1)):
                            if cc < C:
                                sin_bucket(cc)

                # input layer: K=36 -> [256, n]
                for s in streams:
                    s.ps = psh.tile([128, 2, TILE_N], F32, tag="ps", name="psA")
                for s in streams:
                    for mh in range(2):
                        nc.tensor.matmul(
                            s.ps[:, mh, :s.n],
                            winT_sb[:, c, mh * 128:(mh + 1) * 128],
                            s.enc, start=True, stop=True)
                inject_sin(0)
                emit_out(2)
                for s in streams:
                    s.h = hp.tile([128, 2, TILE_N], MMD, tag="h", name="hA")
                    _relu(nc, c, 0, s.h, s.ps, bin_sb[:, c], s.n)

                # mid layers: K=256 -> [256, n]
                for l in range(LMID):
                    for s in streams:
                        s.ps = psh.tile([128, 2, TILE_N], F32, tag="ps",
                                        name="psB")
                    for s in streams:
                        for mh in range(2):
                            for kh in range(2):
                                nc.tensor.matmul(
                                    s.ps[:, mh, :s.n],
                                    wmidT_sb[:, c, l, kh, mh],
                                    s.h[:, kh, :s.n],
                                    start=(kh == 0), stop=(kh == 1))
                    inject_sin(1 + l)
                    if l == 0:
                        emit_out(1)
                    for s in streams:
                        s.h = hp.tile([128, 2, TILE_N], MMD, tag="h", name="hB")
                        _relu(nc, c, 1 + l, s.h, s.ps, bmid_sb[:, c, l], s.n)

                for s in streams:
                    deferred.append((c, s, yc))
            emit_out(len(deferred))

    nc.compile()
    return nc


def _relu(nc, bucket, layer, h, ps, b2, n):
    """h[:, mh, :n] = relu(ps[:, mh, :n] + b2[:, mh]) per the engine plan."""
    for mh in range(2):
        dst, src = h[:, mh, :n], ps[:, mh, :n]
        bias = b2[:, mh:mh + 1]
        if _relu_on_act(bucket, layer, mh):
            nc.scalar.activation(dst, src, AF.Relu, bias=bias)
        else:
            nc.vector.tensor_scalar(dst, src, bias, 0.0, ALU.add, ALU.max)


def _wn(V, g):
    return g[:, None] * V / np.linalg.norm(V, axis=1, keepdims=True)


def kernel(X, cluster_ids, V_in, g_in, b_in, V_mid, g_mid, b_mid,
           V_out, g_out, b_out):
    X = np.ascontiguousarray(np.asarray(X, dtype=np.float32))
    cid = np.asarray(cluster_ids).astype(np.int64)
    N = X.shape[0]

    # ---- host: weight norm (fp32, matches reference) ----
    V_in = np.asarray(V_in, np.float32)
    g_in = np.asarray(g_in, np.float32)
    b_in = np.asarray(b_in, np.float32)
    V_mid = np.asarray(V_mid, np.float32)
    g_mid = np.asarray(g_mid, np.float32)
    b_mid = np.asarray(b_mid, np.float32)
    V_out = np.asarray(V_out, np.float32)
    g_out = np.asarray(g_out, np.float32)
    b_out = np.asarray(b_out, np.float32)

    W_in = np.stack([_wn(V_in[c], g_in[c]) for c in range(C)])          # [C,H,DIN]
    W_mid = np.stack([[_wn(V_mid[c, l], g_mid[c, l]) for l in range(LMID)]
                      for c in range(C)])                               # [C,L,H,H]
    W_out = np.stack([_wn(V_out[c], g_out[c]) for c in range(C)])       # [C,3,H]

    # ---- host: routing ----
    sel = []
    counts = np.zeros((NCORES, C), np.int64)
    for c in range(C):
        ii = np.where(cid == c)[0]
        sel.append([ii[j::NCORES] for j in range(NCORES)])
        for j in range(NCORES):
            counts[j, c] = len(sel[c][j])
    cap = int(counts.max())
    cap = max(64, -(-cap // 64) * 64)  # round up to multiple of 64

    wdt = np.float32 if MM_DTYPE == "f32r" else ml_dtypes.bfloat16
    winT = np.ascontiguousarray(W_in.transpose(2, 0, 1)).astype(wdt)    # [36,C,H]
    binh = np.ascontiguousarray(
        b_in.reshape(C, 2, 128).transpose(2, 0, 1))                     # [128,C,2]
    wmidT = np.ascontiguousarray(
        W_mid.reshape(C, LMID, 2, 128, 2, 128)                          # c,l,mh,mp,kh,kp
        .transpose(5, 0, 1, 4, 2, 3)).astype(wdt)                       # [128,C,L,2,2,128]
    bmidh = np.ascontiguousarray(
        b_mid.reshape(C, LMID, 2, 128).transpose(3, 0, 1, 2))           # [128,C,L,2]
    woutT = np.ascontiguousarray(
        W_out.reshape(C, 3, 2, 128).transpose(3, 0, 2, 1)).astype(wdt)  # [128,C,2,3]
    bouth = np.zeros((128, C), np.float32)  # b_out replicated per col strip
    for base in (0, 32, 64):
        bouth[base:base + 3] = b_out.T

    # ---- host: per-core gathered, range-reduced angles [36, C*cap] ----
    freqs = (2.0 ** np.arange(NFREQ)).astype(np.float64)                # [6]
    in_maps = []
    for j in range(NCORES):
        xrep = np.zeros((DIN, C * cap), np.float32)
        for c in range(C):
            ii = sel[c][j]
            if len(ii) == 0:
                continue
            xg = X[ii].astype(np.float64)                               # [m,3]
            ang = xg[:, None, :] * freqs[None, :, None]                 # [m,6,3]
            ang = np.concatenate([ang, ang + np.pi / 2], axis=2)        # [m,6,6]
            ang = np.mod(ang + np.pi, 2 * np.pi) - np.pi                # [-pi, pi)
            xrep[:, c * cap:c * cap + len(ii)] = \
                ang.reshape(len(ii), DIN).T.astype(np.float32)
        in_maps.append(dict(xrep=xrep, winT=winT, bin=binh, wmidT=wmidT,
                            bmid=bmidh, woutT=woutT, bout=bouth))

    # ---- device ----
    global _last_in_maps
    _last_in_maps = in_maps
    key = (cap, MM_DTYPE)
    if key not in _prog_cache:
        _prog_cache[key] = _build_program(cap)
    nc = _prog_cache[key]
    res = run_bass_kernel_spmd(nc, in_maps, core_ids=list(range(NCORES)))

    # ---- host: scatter back ----
    out = np.zeros((N, 3), np.float32)
    for j in range(NCORES):
        yT = res.results[j]["yT"]                                       # [3, C*cap]
        for c in range(C):
            ii = sel[c][j]
            if len(ii):
                out[ii] = yT[:, c * cap:c * cap + len(ii)].T
    return out
